# revision 47
# baseline (speedup 1.0000x reference)
"""Trainium2 Bass kernel for nn_Attention (B=2, S=2048, D=1024, H=16, causal).

Sharding: head-parallel across 8 NeuronCores — 2 heads per core. Each core:
  1. computes qT/kT/vT for its 2 heads from the full xT (QKV projection,
     transposed layout [128 = 2*hd, S]),
  2. runs causal attention per head with scores in transposed orientation
     (sT[sj, si]) so the PV matmul needs no P transpose; the softmax
     denominator comes free as an extra ones-column in the V operand,
  3. multiplies by its 128-row slice of W_proj producing a partial output
     yT_c [B, D, S].
Host sums the 8 partials, adds b_proj, and transposes back to [B, S, D].

All matmuls run in float32r (full-rate fp32 on the PE; ~1e-4 rounding).
"""
import sys

sys.path.insert(0, "/opt/trn_rl_repo")

import numpy as np
import concourse.bacc as bacc
import concourse.mybir as mybir
import concourse.tile as tile
from concourse.bass_utils import run_bass_kernel_spmd

dt = mybir.dt
F32R = dt.float32r
AF = mybir.ActivationFunctionType

B, S, D, H = 2, 2048, 1024, 16
HD = D // H            # 64
NCORE = 8
HPC = H // NCORE       # 2 heads per core

_CACHE = {}


def build_nc():
    nc = bacc.Bacc("TRN2", target_bir_lowering=False, debug=False)

    xT_d = nc.dram_tensor("xT", [B, D, S], F32R, kind="ExternalInput")
    wq_d = nc.dram_tensor("wq", [128, 8, 128], F32R, kind="ExternalInput")
    wk_d = nc.dram_tensor("wk", [128, 8, 128], F32R, kind="ExternalInput")
    wv_d = nc.dram_tensor("wv", [128, 8, 128], F32R, kind="ExternalInput")
    bq_d = nc.dram_tensor("bq", [128, 1], dt.float32, kind="ExternalInput")
    bk_d = nc.dram_tensor("bk", [128, 1], dt.float32, kind="ExternalInput")
    bv_d = nc.dram_tensor("bv", [128, 1], dt.float32, kind="ExternalInput")
    wp_d = nc.dram_tensor("wp", [128, D], F32R, kind="ExternalInput")
    negm_d = nc.dram_tensor("negm2", [128, 128], dt.float32, kind="ExternalInput")
    id_d = nc.dram_tensor("ident", [128, 128], dt.float32, kind="ExternalInput")
    ones_d = nc.dram_tensor("ones", [128, 64], F32R, kind="ExternalInput")
    zer_d = nc.dram_tensor("zer", [64, S], F32R, kind="ExternalInput")
    yT_d = nc.dram_tensor("yT", [B, D, S], dt.float32, kind="ExternalOutput")

    with tile.TileContext(nc) as tc:
        with (
            tc.tile_pool(name="consts", bufs=1) as consts,
            tc.tile_pool(name="xpool", bufs=32) as xpool,
            tc.tile_pool(name="vpool", bufs=1) as vpool,
            tc.tile_pool(name="qkv", bufs=2) as qkvp,
            tc.tile_pool(name="epool", bufs=3) as epool,
            tc.tile_pool(name="ypool", bufs=4) as ypool,
            tc.tile_pool(name="rpool", bufs=3) as rpool,
            tc.tile_pool(name="ps_mm2", bufs=2, space="PSUM") as ps_mm2,
            tc.tile_pool(name="ps_a", bufs=2, space="PSUM") as ps_a_pool,
            tc.tile_pool(name="ps_aux", bufs=2, space="PSUM") as ps_aux,
        ):
            # ---- constants / weights (once, one DMA each) ----
            wqr = consts.tile([128, 8, 128], F32R, tag="wq")
            wkr = consts.tile([128, 8, 128], F32R, tag="wk")
            wvr = consts.tile([128, 8, 128], F32R, tag="wv")
            nc.scalar.dma_start(wqr[:], wq_d.ap()[:])
            nc.scalar.dma_start(wkr[:], wk_d.ap()[:])
            nc.scalar.dma_start(wvr[:], wv_d.ap()[:])
            wpr = consts.tile([128, D], F32R, tag="wp")
            nc.scalar.dma_start(wpr[:], wp_d.ap()[:])
            bq_sb = consts.tile([128, 1], dt.float32, tag="bq")
            bk_sb = consts.tile([128, 1], dt.float32, tag="bk")
            bv_sb = consts.tile([128, 1], dt.float32, tag="bv")
            nc.gpsimd.dma_start(bq_sb[:], bq_d.ap()[:])
            nc.gpsimd.dma_start(bk_sb[:], bk_d.ap()[:])
            nc.gpsimd.dma_start(bv_sb[:], bv_d.ap()[:])
            negm2 = consts.tile([128, 128], dt.float32, tag="negm2")
            nc.gpsimd.dma_start(negm2[:], negm_d.ap()[:])
            ident = consts.tile([128, 128], dt.float32, tag="ident")
            nc.gpsimd.dma_start(ident[:], id_d.ap()[:])
            ones_r = consts.tile([128, 64], F32R, tag="ones")
            nc.gpsimd.dma_start(ones_r[:], ones_d.ap()[:, :])

            def emit_proj(b, blk, aT, wide=False):
                si0 = 512 * blk
                for dtile in range(8):
                    if wide and dtile % 2 == 1:
                        pool, tag = ps_mm2, "mm2"
                    else:
                        pool, tag = ps_aux, "aux"
                    ps = pool.tile([128, 512], dt.float32, tag=tag,
                                   name=f"psp_{b}_{blk}_{dtile}")
                    nc.tensor.matmul(
                        ps[:],
                        wpr[:, 128 * dtile:128 * (dtile + 1)],
                        aT[:, si0:si0 + 512],
                        start=True,
                        stop=True,
                    )
                    y_sb = ypool.tile([128, 512], dt.float32, tag="y",
                                      name=f"y_{b}_{blk}_{dtile}")
                    nc.vector.tensor_copy(y_sb[:], ps[:])
                    dma_eng = nc.sync if dtile % 2 == 0 else nc.scalar
                    dma_eng.dma_start(
                        yT_d.ap()[
                            b, 128 * dtile:128 * (dtile + 1), si0:si0 + 512,
                        ],
                        y_sb[:],
                    )

            for b in range(B):
                xd = [
                    [
                        xpool.tile([128, 512], F32R, tag="x",
                                   name=f"x_{b}_{d}_{p}")
                        for p in range(4)
                    ]
                    for d in range(8)
                ]
                for bp in range(2):
                    for d in range(8):
                        for t in range(2):
                            p = 2 * bp + t
                            nc.sync.dma_start(
                                xd[d][p][:],
                                xT_d.ap()[b, 128 * d:128 * (d + 1),
                                          512 * p:512 * (p + 1)],
                            )

                qTr = qkvp.tile([128, S], F32R, tag="qT", name=f"qT_{b}")
                kp0 = qkvp.tile([128, S], F32R, tag="kp0", name=f"kp0_{b}")
                kp1 = qkvp.tile([128, S], F32R, tag="kp1", name=f"kp1_{b}")
                nc.gpsimd.dma_start(kp0[64:128, :], zer_d.ap()[:])
                nc.gpsimd.dma_start(kp1[0:64, :], zer_d.ap()[:])
                vT = vpool.tile([128, S], dt.float32, tag="vT", name=f"vT_{b}")

                for bp in range(2):
                    for (w_r, bias, kind) in (
                        (wvr, bv_sb, "v"),
                        (wkr, bk_sb, "k"),
                        (wqr, bq_sb, "q"),
                    ):
                        pp = ps_mm2.tile([128, 2, 512], dt.float32, tag="mm2",
                                         name=f"qkv_{b}_{kind}_{bp}")
                        for d in range(8):
                            for t in range(2):
                                blk = 2 * bp + t
                                nc.tensor.matmul(
                                    pp[:, t, :],
                                    w_r[:, d, :],
                                    xd[d][blk][:],
                                    start=(d == 0),
                                    stop=(d == 7),
                                )
                        cols = slice(1024 * bp, 1024 * (bp + 1))
                        src_ap = pp[:].rearrange("p t f -> p (t f)")
                        with nc.allow_low_precision(reason="f32r qkv bias"):
                            if kind == "q":
                                nc.vector.tensor_scalar_add(
                                    qTr[:, cols], src_ap, bias[:, 0:1])
                            elif kind == "v":
                                nc.vector.tensor_scalar_add(
                                    vT[:, cols], src_ap, bias[:, 0:1])
                            else:
                                nc.vector.tensor_scalar_add(
                                    kp0[0:64, cols], src_ap[0:64, :],
                                    bias[0:64, 0:1])
                                nc.vector.tensor_scalar_add(
                                    kp1[64:128, cols], src_ap[64:128, :],
                                    bias[64:128, 0:1])

                # vhat: v natural per sj tile + ones column, f32r.
                # transposes are emitted inside the attention block that
                # first needs each sj tile, so attention starts early.
                vhat = qkvp.tile([128, 16, 130], F32R, tag="vhat",
                                 name=f"vhat_{b}")
                nc.gpsimd.dma_start(vhat[:, :, 64], ones_d.ap()[:, 0:16])
                nc.gpsimd.dma_start(vhat[:, :, 129], ones_d.ap()[:, 16:32])

                def emit_vhat(j):
                    pst = ps_aux.tile([128, 128], dt.float32, tag="aux",
                                      name=f"tr_{b}_{j}")
                    nc.tensor.transpose(
                        pst[:], vT[:, 128 * j:128 * (j + 1)], ident[:]
                    )
                    nc.vector.tensor_copy(vhat[:, j, 0:64], pst[:, 0:64])
                    nc.vector.tensor_copy(vhat[:, j, 65:129], pst[:, 64:128])

                aT = qkvp.tile([128, S], F32R, tag="aT", name=f"aT_{b}")

                # ---- causal attention, heads paired in adjacent PSUM banks --
                for blk in range(4):
                    si0 = 512 * blk
                    jlast = 4 * blk + 3
                    for j in range(4 * blk, 4 * blk + 4):
                        emit_vhat(j)
                    psa = [
                        ps_a_pool.tile([65, 512], dt.float32, tag="acc",
                                       name=f"psa_{b}_{blk}_{hl}")
                        for hl in range(HPC)
                    ]
                    for j in range(jlast + 1):
                        off = max(0, 128 * (j - 4 * blk))
                        w = 512 - off
                        pp = ps_mm2.tile([128, 2, 512], dt.float32, tag="mm2",
                                         name=f"pp_{b}_{blk}_{j}")
                        for hl, kp in ((0, kp0), (1, kp1)):
                            nc.tensor.matmul(
                                pp[:, hl, 0:w],
                                kp[:, 128 * j:128 * (j + 1)],
                                qTr[:, si0 + off:si0 + 512],
                                start=True,
                                stop=True,
                            )
                        ee = epool.tile([128, 2, 512], F32R, tag="eT",
                                        name=f"ee_{b}_{blk}_{j}")
                        nc.scalar.activation(
                            ee[:, :, 0:w], pp[:, :, 0:w], AF.Exp, scale=0.125
                        )
                        if j >= 4 * blk:
                            with nc.allow_low_precision(reason="causal mask"):
                                for hl in range(HPC):
                                    nc.vector.tensor_mul(
                                        ee[:, hl, 0:128], ee[:, hl, 0:128],
                                        negm2[:]
                                    )
                        for hl in range(HPC):
                            nc.tensor.matmul(
                                psa[hl][:, off:512],
                                vhat[:, j, 65 * hl:65 * hl + 65],
                                ee[:, hl, 0:w],
                                start=(j == 0),
                                stop=(j == jlast),
                            )
                    if blk > 0:
                        emit_proj(b, blk - 1, aT)
                    for hl in range(HPC):
                        p0 = 64 * hl
                        a_sb = rpool.tile([65, 512], F32R, tag="a_sb",
                                          name=f"asb_{b}_{blk}_{hl}")
                        nc.vector.tensor_copy(a_sb[:], psa[hl][:])
                        lnl = rpool.tile([1, 512], F32R, tag="lnl",
                                         name=f"lnl_{b}_{blk}_{hl}")
                        nc.scalar.activation(lnl[:], a_sb[64:65, :], AF.Ln)
                        psb = ps_aux.tile([64, 512], dt.float32, tag="aux",
                                          name=f"psb_{b}_{blk}_{hl}")
                        nc.tensor.matmul(
                            psb[:], ones_r[0:1, :], lnl[:],
                            start=True, stop=True
                        )
                        rec_sb = rpool.tile([64, 512], dt.float32, tag="rec_sb",
                                            name=f"recs_{b}_{blk}_{hl}")
                        nc.scalar.activation(rec_sb[:], psb[:], AF.Exp,
                                             scale=-1.0)
                        with nc.allow_low_precision(reason="f32r attn normalize"):
                            nc.vector.tensor_mul(
                                aT[p0:p0 + 64, si0:si0 + 512],
                                a_sb[0:64, :],
                                rec_sb[:],
                            )
                    if blk == 3:
                        emit_proj(b, 3, aT)
    nc.compile()
    return nc


def _get_nc():
    if "nc" not in _CACHE:
        _CACHE["nc"] = build_nc()
    return _CACHE["nc"]


def prep_w(w):
    # [1024, 128] -> [128(p), 8(d), 128(m)] so the SBUF load is contiguous
    return np.ascontiguousarray(w.reshape(8, 128, 128).transpose(1, 0, 2))


def make_in_maps(x, W_attn, b_attn, W_proj):
    x = np.ascontiguousarray(x, dtype=np.float32)
    xT = np.ascontiguousarray(x.transpose(0, 2, 1))

    p = np.arange(128)
    negm = np.where(p[:, None] <= p[None, :], 1.0, 0.0).astype(np.float32)
    negm2 = negm
    ident = np.eye(128, dtype=np.float32)
    ones = np.ones((128, 64), np.float32)

    in_maps = []
    for c in range(NCORE):
        col0 = HD * HPC * c
        in_maps.append({
            "xT": xT,
            "wq": prep_w(W_attn[:, col0:col0 + 128]),
            "wk": prep_w(W_attn[:, D + col0:D + col0 + 128]),
            "wv": prep_w(W_attn[:, 2 * D + col0:2 * D + col0 + 128]),
            "bq": np.ascontiguousarray(b_attn[col0:col0 + 128].reshape(128, 1)),
            "bk": np.ascontiguousarray(b_attn[D + col0:D + col0 + 128].reshape(128, 1)),
            "bv": np.ascontiguousarray(b_attn[2 * D + col0:2 * D + col0 + 128].reshape(128, 1)),
            "wp": np.ascontiguousarray(W_proj[128 * c:128 * (c + 1), :]),
            "negm2": negm2,
            "zer": np.zeros((64, S), np.float32),
            "ident": ident,
            "ones": ones,
        })
    return in_maps


def gather(results, b_proj):
    acc = np.zeros((B, D, S), np.float64)
    for r in results:
        acc += r["yT"]
    out = acc.transpose(0, 2, 1) + np.asarray(b_proj, np.float64)[None, None, :]
    return np.ascontiguousarray(out.astype(np.float32))


def kernel(x, W_attn, b_attn, W_proj, b_proj, _trace=False, _trace_kwargs=None):
    nc = _get_nc()
    in_maps = make_in_maps(np.asarray(x), np.asarray(W_attn),
                           np.asarray(b_attn), np.asarray(W_proj))
    res = run_bass_kernel_spmd(
        nc, in_maps, list(range(NCORE)), trace=_trace, **(_trace_kwargs or {})
    )
    out = gather(res.results, np.asarray(b_proj))
    if _trace:
        kernel.last_result = res
    return out


# revision 48
# speedup vs baseline: 1.0611x; 1.0611x over previous
"""Trainium2 Bass kernel for nn_Attention (B=2, S=2048, D=1024, H=16, causal).

Sharding: head-parallel across 8 NeuronCores — 2 heads per core. Each core:
  1. computes qT/kT/vT for its 2 heads from the full xT (QKV projection,
     transposed layout [128 = 2*hd, S]),
  2. runs causal attention per head with scores in transposed orientation
     (sT[sj, si]) so the PV matmul needs no P transpose; the softmax
     denominator comes free as an extra ones-column in the V operand,
  3. multiplies by its 128-row slice of W_proj producing a partial output
     yT_c [B, D, S].
Host sums the 8 partials, adds b_proj, and transposes back to [B, S, D].

All matmuls run in float32r (full-rate fp32 on the PE; ~1e-4 rounding).
"""
import sys

sys.path.insert(0, "/opt/trn_rl_repo")

import numpy as np
import concourse.bacc as bacc
import concourse.mybir as mybir
import concourse.tile as tile
from concourse.bass_utils import run_bass_kernel_spmd

dt = mybir.dt
F32R = dt.float32r
AF = mybir.ActivationFunctionType

B, S, D, H = 2, 2048, 1024, 16
HD = D // H            # 64
NCORE = 8
HPC = H // NCORE       # 2 heads per core

_CACHE = {}


def build_nc():
    nc = bacc.Bacc("TRN2", target_bir_lowering=False, debug=False)

    xT_d = nc.dram_tensor("xT", [B, D, S], F32R, kind="ExternalInput")
    wq_d = nc.dram_tensor("wq", [128, 8, 128], F32R, kind="ExternalInput")
    wk_d = nc.dram_tensor("wk", [128, 8, 128], F32R, kind="ExternalInput")
    wv_d = nc.dram_tensor("wv", [128, 8, 128], F32R, kind="ExternalInput")
    bq_d = nc.dram_tensor("bq", [128, 1], dt.float32, kind="ExternalInput")
    bk_d = nc.dram_tensor("bk", [128, 1], dt.float32, kind="ExternalInput")
    bv_d = nc.dram_tensor("bv", [128, 1], dt.float32, kind="ExternalInput")
    wp_d = nc.dram_tensor("wp", [128, D], F32R, kind="ExternalInput")
    negm_d = nc.dram_tensor("negm2", [128, 128], dt.float32, kind="ExternalInput")
    id_d = nc.dram_tensor("ident", [128, 128], dt.float32, kind="ExternalInput")
    ones_d = nc.dram_tensor("ones", [128, 64], F32R, kind="ExternalInput")
    zer_d = nc.dram_tensor("zer", [64, S], F32R, kind="ExternalInput")
    yT_d = nc.dram_tensor("yT", [B, D, S], dt.float32, kind="ExternalOutput")

    with tile.TileContext(nc) as tc:
        with (
            tc.tile_pool(name="consts", bufs=1) as consts,
            tc.tile_pool(name="xpool", bufs=32) as xpool,
            tc.tile_pool(name="vpool", bufs=1) as vpool,
            tc.tile_pool(name="qkv", bufs=2) as qkvp,
            tc.tile_pool(name="epool", bufs=3) as epool,
            tc.tile_pool(name="ypool", bufs=4) as ypool,
            tc.tile_pool(name="rpool", bufs=3) as rpool,
            tc.tile_pool(name="ps_mm2", bufs=2, space="PSUM") as ps_mm2,
            tc.tile_pool(name="ps_a", bufs=2, space="PSUM") as ps_a_pool,
            tc.tile_pool(name="ps_aux", bufs=2, space="PSUM") as ps_aux,
        ):
            # ---- constants / weights (once, one DMA each) ----
            wqr = consts.tile([128, 8, 128], F32R, tag="wq")
            wkr = consts.tile([128, 8, 128], F32R, tag="wk")
            wvr = consts.tile([128, 8, 128], F32R, tag="wv")
            nc.scalar.dma_start(wqr[:], wq_d.ap()[:])
            nc.scalar.dma_start(wkr[:], wk_d.ap()[:])
            nc.scalar.dma_start(wvr[:], wv_d.ap()[:])
            wpr = consts.tile([128, D], F32R, tag="wp")
            nc.scalar.dma_start(wpr[:], wp_d.ap()[:])
            bq_sb = consts.tile([128, 1], dt.float32, tag="bq")
            bk_sb = consts.tile([128, 1], dt.float32, tag="bk")
            bv_sb = consts.tile([128, 1], dt.float32, tag="bv")
            nc.gpsimd.dma_start(bq_sb[:], bq_d.ap()[:])
            nc.gpsimd.dma_start(bk_sb[:], bk_d.ap()[:])
            nc.gpsimd.dma_start(bv_sb[:], bv_d.ap()[:])
            negm2 = consts.tile([128, 128], dt.float32, tag="negm2")
            nc.gpsimd.dma_start(negm2[:], negm_d.ap()[:])
            ident = consts.tile([128, 128], dt.float32, tag="ident")
            nc.gpsimd.dma_start(ident[:], id_d.ap()[:])
            ones_r = consts.tile([128, 64], F32R, tag="ones")
            nc.gpsimd.dma_start(ones_r[:], ones_d.ap()[:, :])

            def emit_proj(b, blk, aT, wide=False):
                si0 = 512 * blk
                for dtile in range(8):
                    if wide and dtile % 2 == 1:
                        pool, tag = ps_mm2, "mm2"
                    else:
                        pool, tag = ps_aux, "aux"
                    ps = pool.tile([128, 512], dt.float32, tag=tag,
                                   name=f"psp_{b}_{blk}_{dtile}")
                    nc.tensor.matmul(
                        ps[:],
                        wpr[:, 128 * dtile:128 * (dtile + 1)],
                        aT[:, si0:si0 + 512],
                        start=True,
                        stop=True,
                    )
                    y_sb = ypool.tile([128, 512], dt.float32, tag="y",
                                      name=f"y_{b}_{blk}_{dtile}")
                    nc.vector.tensor_copy(y_sb[:], ps[:])
                    dma_eng = nc.sync if dtile % 2 == 0 else nc.scalar
                    dma_eng.dma_start(
                        yT_d.ap()[
                            b, 128 * dtile:128 * (dtile + 1), si0:si0 + 512,
                        ],
                        y_sb[:],
                    )

            for b in range(B):
                xd = [
                    [
                        xpool.tile([128, 512], F32R, tag="x",
                                   name=f"x_{b}_{d}_{p}")
                        for p in range(4)
                    ]
                    for d in range(8)
                ]
                for bp in range(2):
                    for d in range(8):
                        for t in range(2):
                            p = 2 * bp + t
                            nc.sync.dma_start(
                                xd[d][p][:],
                                xT_d.ap()[b, 128 * d:128 * (d + 1),
                                          512 * p:512 * (p + 1)],
                            )

                qTr = qkvp.tile([128, S], F32R, tag="qT", name=f"qT_{b}")
                kp0 = qkvp.tile([128, S], F32R, tag="kp0", name=f"kp0_{b}")
                kp1 = qkvp.tile([128, S], F32R, tag="kp1", name=f"kp1_{b}")
                nc.gpsimd.dma_start(kp0[64:128, :], zer_d.ap()[:])
                nc.gpsimd.dma_start(kp1[0:64, :], zer_d.ap()[:])
                vT = vpool.tile([128, S], dt.float32, tag="vT", name=f"vT_{b}")

                for bp in range(2):
                    for (w_r, bias, kind) in (
                        (wvr, bv_sb, "v"),
                        (wkr, bk_sb, "k"),
                        (wqr, bq_sb, "q"),
                    ):
                        pp = ps_mm2.tile([128, 2, 512], dt.float32, tag="mm2",
                                         name=f"qkv_{b}_{kind}_{bp}")
                        for d in range(8):
                            for t in range(2):
                                blk = 2 * bp + t
                                nc.tensor.matmul(
                                    pp[:, t, :],
                                    w_r[:, d, :],
                                    xd[d][blk][:],
                                    start=(d == 0),
                                    stop=(d == 7),
                                )
                        cols = slice(1024 * bp, 1024 * (bp + 1))
                        src_ap = pp[:].rearrange("p t f -> p (t f)")
                        with nc.allow_low_precision(reason="f32r qkv bias"):
                            if kind == "q":
                                nc.vector.tensor_scalar_add(
                                    qTr[:, cols], src_ap, bias[:, 0:1])
                            elif kind == "v":
                                nc.vector.tensor_scalar_add(
                                    vT[:, cols], src_ap, bias[:, 0:1])
                            else:
                                nc.vector.tensor_scalar_add(
                                    kp0[0:64, cols], src_ap[0:64, :],
                                    bias[0:64, 0:1])
                                nc.vector.tensor_scalar_add(
                                    kp1[64:128, cols], src_ap[64:128, :],
                                    bias[64:128, 0:1])

                # vhat: v natural per sj tile + ones column, f32r.
                # transposes are emitted inside the attention block that
                # first needs each sj tile, so attention starts early.
                vhat = qkvp.tile([128, 16, 130], F32R, tag="vhat",
                                 name=f"vhat_{b}")
                nc.gpsimd.dma_start(vhat[:, :, 64], ones_d.ap()[:, 0:16])
                nc.gpsimd.dma_start(vhat[:, :, 129], ones_d.ap()[:, 16:32])

                def emit_vhat(j):
                    pst = ps_aux.tile([128, 128], dt.float32, tag="aux",
                                      name=f"tr_{b}_{j}")
                    nc.tensor.transpose(
                        pst[:], vT[:, 128 * j:128 * (j + 1)], ident[:]
                    )
                    nc.vector.tensor_copy(vhat[:, j, 0:64], pst[:, 0:64])
                    nc.vector.tensor_copy(vhat[:, j, 65:129], pst[:, 64:128])

                aT = qkvp.tile([128, S], F32R, tag="aT", name=f"aT_{b}")

                # ---- causal attention, heads paired in adjacent PSUM banks --
                for blk in range(4):
                    si0 = 512 * blk
                    jlast = 4 * blk + 3
                    for j in range(4 * blk, 4 * blk + 4):
                        emit_vhat(j)
                    psa = [
                        ps_a_pool.tile([65, 512], dt.float32, tag="acc",
                                       name=f"psa_{b}_{blk}_{hl}")
                        for hl in range(HPC)
                    ]
                    for j in range(jlast + 1):
                        off = max(0, 128 * (j - 4 * blk))
                        w = 512 - off
                        pp = ps_mm2.tile([128, 2, 512], dt.float32, tag="mm2",
                                         name=f"pp_{b}_{blk}_{j}")
                        for hl, kp in ((0, kp0), (1, kp1)):
                            nc.tensor.matmul(
                                pp[:, hl, 0:w],
                                kp[:, 128 * j:128 * (j + 1)],
                                qTr[:, si0 + off:si0 + 512],
                                start=True,
                                stop=True,
                            )
                        ee = epool.tile([128, 2, 512], F32R, tag="eT",
                                        name=f"ee_{b}_{blk}_{j}")
                        nc.scalar.activation(
                            ee[:, :, 0:w], pp[:, :, 0:w], AF.Exp, scale=0.125
                        )
                        if j >= 4 * blk:
                            with nc.allow_low_precision(reason="causal mask"):
                                for hl in range(HPC):
                                    nc.vector.tensor_mul(
                                        ee[:, hl, 0:128], ee[:, hl, 0:128],
                                        negm2[:]
                                    )
                        for hl in range(HPC):
                            nc.tensor.matmul(
                                psa[hl][:, off:512],
                                vhat[:, j, 65 * hl:65 * hl + 65],
                                ee[:, hl, 0:w],
                                start=(j == 0),
                                stop=(j == jlast),
                            )
                    if blk > 0:
                        emit_proj(b, blk - 1, aT)
                    for hl in range(HPC):
                        p0 = 64 * hl
                        a_sb = rpool.tile([65, 512], F32R, tag="a_sb",
                                          name=f"asb_{b}_{blk}_{hl}")
                        nc.vector.tensor_copy(a_sb[:], psa[hl][:])
                        lnl = rpool.tile([1, 512], F32R, tag="lnl",
                                         name=f"lnl_{b}_{blk}_{hl}")
                        nc.scalar.activation(lnl[:], psa[hl][64:65, :], AF.Ln)
                        psb = ps_aux.tile([64, 512], dt.float32, tag="aux",
                                          name=f"psb_{b}_{blk}_{hl}")
                        nc.tensor.matmul(
                            psb[:], ones_r[0:1, :], lnl[:],
                            start=True, stop=True
                        )
                        rec_sb = rpool.tile([64, 512], dt.float32, tag="rec_sb",
                                            name=f"recs_{b}_{blk}_{hl}")
                        nc.scalar.activation(rec_sb[:], psb[:], AF.Exp,
                                             scale=-1.0)
                        with nc.allow_low_precision(reason="f32r attn normalize"):
                            nc.vector.tensor_mul(
                                aT[p0:p0 + 64, si0:si0 + 512],
                                a_sb[0:64, :],
                                rec_sb[:],
                            )
                    if blk == 3:
                        emit_proj(b, 3, aT)
    nc.compile()
    return nc


def _get_nc():
    if "nc" not in _CACHE:
        _CACHE["nc"] = build_nc()
    return _CACHE["nc"]


def prep_w(w):
    # [1024, 128] -> [128(p), 8(d), 128(m)] so the SBUF load is contiguous
    return np.ascontiguousarray(w.reshape(8, 128, 128).transpose(1, 0, 2))


def make_in_maps(x, W_attn, b_attn, W_proj):
    x = np.ascontiguousarray(x, dtype=np.float32)
    xT = np.ascontiguousarray(x.transpose(0, 2, 1))

    p = np.arange(128)
    negm = np.where(p[:, None] <= p[None, :], 1.0, 0.0).astype(np.float32)
    negm2 = negm
    ident = np.eye(128, dtype=np.float32)
    ones = np.ones((128, 64), np.float32)

    in_maps = []
    for c in range(NCORE):
        col0 = HD * HPC * c
        in_maps.append({
            "xT": xT,
            "wq": prep_w(W_attn[:, col0:col0 + 128]),
            "wk": prep_w(W_attn[:, D + col0:D + col0 + 128]),
            "wv": prep_w(W_attn[:, 2 * D + col0:2 * D + col0 + 128]),
            "bq": np.ascontiguousarray(b_attn[col0:col0 + 128].reshape(128, 1)),
            "bk": np.ascontiguousarray(b_attn[D + col0:D + col0 + 128].reshape(128, 1)),
            "bv": np.ascontiguousarray(b_attn[2 * D + col0:2 * D + col0 + 128].reshape(128, 1)),
            "wp": np.ascontiguousarray(W_proj[128 * c:128 * (c + 1), :]),
            "negm2": negm2,
            "zer": np.zeros((64, S), np.float32),
            "ident": ident,
            "ones": ones,
        })
    return in_maps


def gather(results, b_proj):
    acc = np.zeros((B, D, S), np.float64)
    for r in results:
        acc += r["yT"]
    out = acc.transpose(0, 2, 1) + np.asarray(b_proj, np.float64)[None, None, :]
    return np.ascontiguousarray(out.astype(np.float32))


def kernel(x, W_attn, b_attn, W_proj, b_proj, _trace=False, _trace_kwargs=None):
    nc = _get_nc()
    in_maps = make_in_maps(np.asarray(x), np.asarray(W_attn),
                           np.asarray(b_attn), np.asarray(W_proj))
    res = run_bass_kernel_spmd(
        nc, in_maps, list(range(NCORE)), trace=_trace, **(_trace_kwargs or {})
    )
    out = gather(res.results, np.asarray(b_proj))
    if _trace:
        kernel.last_result = res
    return out
